# revision 1
# baseline (speedup 1.0000x reference)
"""Causal self-attention kernel for 8 Trainium2 NeuronCores.

Problem: B=2, T=2048, D=2048, H=16, Dh=128, fp32.
  qkv = x @ Wqkv + bqkv ; per-head causal attention ; out = att @ Wout + bout

Sharding (tensor parallel over heads + AllToAll before out_proj):
  Core c owns heads {2c, 2c+1}. Each core computes, for all 4096 tokens,
  Q^T/K^T (head-dim on partitions) and V (token-dim on partitions) for its
  two heads via the QKV projection with its 768-column shard of Wqkv, runs
  causal attention locally (scores are computed transposed: S^T[k,q], so
  the softmax reduction over k maps to an all-ones matmul on the partition
  axis which also broadcasts the denominator), and produces att^T
  [256, 2048] per batch. Four small AllToAlls (one per half-batch of
  tokens) redistribute from head-sharded to token-sharded; core c then
  projects its 128-token slices with the full Wout.

  Batch 0's attention is emitted interleaved with batch 1's projection so
  the PE fills the exp-latency gaps; the early AllToAlls and batch-0's
  output projection overlap batch 1's attention.

All matmuls run in float32r (full PE rate at free-dim >= 256, ~1e-4 rel
error). PSUM accumulation is fp32.
"""

import numpy as np

import concourse.bass as bass
import concourse.mybir as mybir
import concourse.tile as tile
from concourse import bacc
from concourse.bass_utils import run_bass_kernel_spmd

B, T, D, H, Dh = 2, 2048, 2048, 16, 128
NT = B * T                  # 4096 tokens total
W = 8                       # cores
HL = H // W                 # 2 heads per core
CQKV = 3 * HL * Dh          # 768 qkv columns per core
KO = D // 128               # 16 contraction subtiles
TC = 256                    # token chunk for projection rhs
NTC_B = T // TC             # 8 chunks per batch
QC = 512                    # attention q-chunk
NQC = T // QC               # 4 q-chunks per batch
HT = T // 2                 # half-batch token span (one AllToAll each)
TOKH = HT // W              # 128 tokens per core per half-batch exchange
SCALE = 1.0 / float(np.sqrt(Dh))

F32 = mybir.dt.float32
F32R = mybir.dt.float32r


def _build():
    nc = bacc.Bacc("TRN2", target_bir_lowering=False, debug=False,
                   enable_asserts=True, num_devices=W)
    xT = nc.dram_tensor("xT", [D, NT], F32, kind="ExternalInput").ap()
    wqkv = nc.dram_tensor("wqkv", [D, CQKV], F32, kind="ExternalInput").ap()
    bqkv = nc.dram_tensor("bqkv", [CQKV], F32, kind="ExternalInput").ap()
    wout = nc.dram_tensor("wout", [D, D], F32, kind="ExternalInput").ap()
    masktri = nc.dram_tensor("masktri", [128, 128], F32, kind="ExternalInput").ap()
    ones = nc.dram_tensor("ones", [128, 128], F32, kind="ExternalInput").ap()
    bvbc = nc.dram_tensor("bvbc", [128, HL * Dh], F32, kind="ExternalInput").ap()
    boutbc = nc.dram_tensor("boutbc", [128, D], F32, kind="ExternalInput").ap()
    # rows [(b*2+half)*TOKH ...): tokens [half*HT + c*TOKH ...) of batch b
    out = nc.dram_tensor("out", [B * 2 * TOKH, D], F32, kind="ExternalOutput").ap()

    xT_v = xT.rearrange("(ko p) t -> p ko t", p=128)
    wqkv_v = wqkv.rearrange("(ko p) c -> p ko c", p=128)
    wout_v = wout.rearrange("(ko p) c -> p ko c", p=128)

    with tile.TileContext(nc) as tc:
        with tc.tile_pool(name="persist", bufs=1) as persist, \
             tc.tile_pool(name="dram", bufs=1, space="DRAM") as dram_pool:
            mask_sb = persist.tile([128, 128], F32R)
            ones_sb = persist.tile([128, 128], F32R)
            bqk_sb = persist.tile([128, 2 * HL], F32)      # Q,K bias (col on partition)
            bv_sb = persist.tile([128, HL * Dh], F32)      # V bias pre-broadcast

            nc.sync.dma_start(mask_sb[:], masktri.bitcast(F32R))
            nc.sync.dma_start(ones_sb[:], ones.bitcast(F32R))
            nc.sync.dma_start(bqk_sb[:], bqkv[0:2 * HL * 128].rearrange("(cc p) -> p cc", p=128))
            nc.sync.dma_start(bv_sb[:], bvbc)

            a2a_in = [[dram_pool.tile([W, HL * 128, TOKH], F32, name=f"a2a_in{b}{h}")
                       for h in range(2)] for b in range(B)]
            a2a_out = [[dram_pool.tile([W, HL * 128, TOKH], F32, name=f"a2a_out{b}{h}")
                        for h in range(2)] for b in range(B)]

            def alloc_qkv(pool):
                qT = pool.tile([128, HL, T], F32R, name="qT")
                kT = pool.tile([128, HL, T], F32R, name="kT")
                v = pool.tile([128, HL, T // 128, Dh], F32R, name="v")
                return qT, kT, v

            def emit_proj_chunk(qkv, wqkv_sb, x_pool, proj_psum, b, tci):
                """Project one 256-token chunk of batch b into (qT, kT, v)."""
                qT_sb, kT_sb, v_sb = qkv
                t0 = b * T + tci * TC
                x_sb = x_pool.tile([128, KO, TC], F32R, name="x_sb")
                nc.sync.dma_start(x_sb[:], xT_v[:, :, t0:t0 + TC].bitcast(F32R))
                for cc in range(2 * HL):
                    ps = proj_psum.tile([128, TC], F32, name="proj_ps")
                    for ko in range(KO):
                        nc.tensor.matmul(
                            ps[:], wqkv_sb[ko][:, cc * 128:(cc + 1) * 128],
                            x_sb[:, ko, :], start=(ko == 0), stop=(ko == KO - 1))
                    dest = qT_sb if cc < HL else kT_sb
                    hl = cc if cc < HL else cc - HL
                    nc.vector.tensor_scalar_add(
                        dest[:, hl, tci * TC:(tci + 1) * TC], ps[:],
                        bqk_sb[:, cc:cc + 1])
                for tb in range(TC // 128):
                    ps = proj_psum.tile([128, HL * Dh], F32, name="proj_ps")
                    for ko in range(KO):
                        nc.tensor.matmul(
                            ps[:], x_sb[:, ko, tb * 128:(tb + 1) * 128],
                            wqkv_sb[ko][:, 2 * HL * 128:], start=(ko == 0), stop=(ko == KO - 1))
                    idx = tci * (TC // 128) + tb
                    nc.vector.tensor_tensor(
                        v_sb[:, :, idx, :],
                        ps[:].rearrange("p (hl d) -> p hl d", hl=HL),
                        bv_sb[:].rearrange("p (hl d) -> p hl d", hl=HL),
                        mybir.AluOpType.add)

            def emit_attn_group(qkv, att_sb, pools, hl, qc):
                """One (head, q-chunk) attention group: S^T -> exp -> P^T V.

                k-blocks are processed in pairs sharing one 2-bank PSUM tile
                so off-diagonal pairs need a single exp over 1024 columns.
                """
                qT_sb, kT_sb, v_sb = qkv
                ex_pool, rden_pool, s_psum, av_psum, d_psum = pools
                q0 = qc * QC
                nkb = (qc + 1) * (QC // 128)
                ps_av = av_psum.tile([128, QC], F32, name="ps_av")
                ps_dbc = d_psum.tile([128, QC], F32, name="ps_dbc")
                for kbp in range(nkb // 2):
                    kbs = (2 * kbp, 2 * kbp + 1)
                    os_ = [kb - qc * (QC // 128) for kb in kbs]
                    vss = [max(0, o) * 128 for o in os_]
                    ps_s2 = s_psum.tile([128, 2, QC], F32, name="ps_s2")
                    ex2 = ex_pool.tile([128, 2, QC], F32R, name="ex2")
                    for i, kb in enumerate(kbs):
                        nc.tensor.matmul(
                            ps_s2[:, i, vss[i]:], kT_sb[:, hl, kb * 128:(kb + 1) * 128],
                            qT_sb[:, hl, q0 + vss[i]:q0 + QC], start=True, stop=True)
                    if vss[0] == 0 and vss[1] == 0:
                        nc.scalar.activation(
                            ex2[:], ps_s2[:], mybir.ActivationFunctionType.Exp,
                            scale=SCALE)
                    else:
                        for i in range(2):
                            nc.scalar.activation(
                                ex2[:, i, vss[i]:], ps_s2[:, i, vss[i]:],
                                mybir.ActivationFunctionType.Exp, scale=SCALE)
                    for i, kb in enumerate(kbs):
                        if os_[i] >= 0:
                            nc.vector.tensor_tensor(
                                ex2[:, i, vss[i]:vss[i] + 128],
                                ex2[:, i, vss[i]:vss[i] + 128], mask_sb[:],
                                mybir.AluOpType.mult)
                        nc.tensor.matmul(
                            ps_av[:, vss[i]:], v_sb[:, hl, kb, :], ex2[:, i, vss[i]:],
                            start=(kb == 0), stop=(kb == nkb - 1))
                        nc.tensor.matmul(
                            ps_dbc[:, vss[i]:], ones_sb[:], ex2[:, i, vss[i]:],
                            start=(kb == 0), stop=(kb == nkb - 1))
                rden = rden_pool.tile([128, QC], F32, name="rden")
                nc.vector.reciprocal(rden[:], ps_dbc[:])
                nc.vector.tensor_tensor(
                    att_sb[:, hl, q0:q0 + QC], ps_av[:], rden[:],
                    mybir.AluOpType.mult)

            def emit_a2a(att_sb, b, half):
                for r in range(W):
                    nc.gpsimd.dma_start(
                        a2a_in[b][half][r].rearrange("(hl p) t -> p hl t", hl=HL, p=128),
                        att_sb[:, :, half * HT + r * TOKH:half * HT + (r + 1) * TOKH])
                nc.gpsimd.collective_compute(
                    "AllToAll", mybir.AluOpType.bypass,
                    replica_groups=[list(range(W))],
                    ins=[a2a_in[b][half][:].opt()], outs=[a2a_out[b][half][:].opt()])

            def emit_outproj(attall_pool, wout_pool, o_pool, out_psum, bout_sb, b):
                attall = []
                for half in range(2):
                    attall_sb = attall_pool.tile([128, KO, TOKH], F32R, name="attall")
                    nc.sync.dma_start(
                        attall_sb[:],
                        a2a_out[b][half][:].rearrange(
                            "r (x p) t -> p (r x) t", x=HL, p=128).bitcast(F32R))
                    attall.append(attall_sb)
                for colc in range(D // 512):
                    wout_sb = wout_pool.tile([128, KO, 512], F32R, name="wout_sb")
                    nc.sync.dma_start(
                        wout_sb[:], wout_v[:, :, colc * 512:(colc + 1) * 512].bitcast(F32R))
                    for half in (1, 0):
                        ps_o = out_psum.tile([128, 512], F32, name="ps_o")
                        for ko in range(KO):
                            nc.tensor.matmul(
                                ps_o[:], attall[half][:, ko, :],
                                wout_sb[:, ko, :], start=(ko == 0), stop=(ko == KO - 1))
                        o_sb = o_pool.tile([128, 512], F32, name="o_sb")
                        nc.vector.tensor_tensor(
                            o_sb[:], ps_o[:],
                            bout_sb[:, colc * 512:(colc + 1) * 512],
                            mybir.AluOpType.add)
                        nc.sync.dma_start(
                            out[(b * 2 + half) * TOKH:(b * 2 + half + 1) * TOKH,
                                colc * 512:(colc + 1) * 512],
                            o_sb[:])

            # heavy half (qc 2,3) first so the last A2A covers the small half
            groups_h0 = [(hl, qc) for qc in (1, 0) for hl in range(HL)]
            groups_h1 = [(hl, qc) for qc in (3, 2) for hl in range(HL)]

            with tc.tile_pool(name="qkv1_pool", bufs=1) as qkv1_pool:
                qkv1 = alloc_qkv(qkv1_pool)
                with tc.tile_pool(name="qkv0_pool", bufs=1) as qkv0_pool:
                    qkv0 = alloc_qkv(qkv0_pool)
                    with tc.tile_pool(name="att0_pool", bufs=1) as att0_pool:
                        att0_sb = att0_pool.tile([128, HL, T], F32)
                        with tc.tile_pool(name="wq_pool", bufs=1) as wq_pool, \
                             tc.tile_pool(name="x_pool", bufs=2) as x_pool, \
                             tc.tile_pool(name="proj_psum", bufs=2, space="PSUM") as proj_psum, \
                             tc.tile_pool(name="ex0_pool", bufs=2) as ex0_pool, \
                             tc.tile_pool(name="rden0_pool", bufs=1) as rden0_pool, \
                             tc.tile_pool(name="s0_psum", bufs=2, space="PSUM") as s0_psum, \
                             tc.tile_pool(name="av0_psum", bufs=1, space="PSUM") as av0_psum, \
                             tc.tile_pool(name="d0_psum", bufs=1, space="PSUM") as d0_psum:
                            wqkv_sb = [wq_pool.tile([128, CQKV], F32R,
                                                     name=f"wqkv{ko}", bufs=1)
                                       for ko in range(KO)]
                            for ko in range(KO):
                                nc.sync.dma_start(
                                    wqkv_sb[ko][:],
                                    wqkv_v[:, ko, :].bitcast(F32R))
                            pools0 = (ex0_pool, rden0_pool, s0_psum, av0_psum, d0_psum)
                            # batch-0 projection
                            for tci in range(NTC_B):
                                emit_proj_chunk(qkv0, wqkv_sb, x_pool, proj_psum, 0, tci)
                            # batch-1 projection interleaved with batch-0 attention
                            groups0 = groups_h1 + groups_h0
                            for i in range(NTC_B):
                                emit_proj_chunk(qkv1, wqkv_sb, x_pool, proj_psum, 1, i)
                                emit_attn_group(qkv0, att0_sb, pools0, *groups0[i])
                                if i == NTC_B // 2 - 1:
                                    emit_a2a(att0_sb, 0, 1)
                        emit_a2a(att0_sb, 0, 0)
                # batch-1 attention overlapping A2As and batch-0 out-proj
                with tc.tile_pool(name="att1_pool", bufs=1) as att1_pool:
                    att1_sb = att1_pool.tile([128, HL, T], F32)
                    with tc.tile_pool(name="ex1_pool", bufs=3) as ex1_pool, \
                         tc.tile_pool(name="rden1_pool", bufs=2) as rden1_pool, \
                         tc.tile_pool(name="s1_psum", bufs=2, space="PSUM") as s1_psum, \
                         tc.tile_pool(name="av1_psum", bufs=2, space="PSUM") as av1_psum, \
                         tc.tile_pool(name="d1_psum", bufs=1, space="PSUM") as d1_psum, \
                         tc.tile_pool(name="attall_pool", bufs=4) as attall_pool, \
                         tc.tile_pool(name="wout_pool", bufs=2) as wout_pool, \
                         tc.tile_pool(name="o_pool", bufs=3) as o_pool, \
                         tc.tile_pool(name="out_psum", bufs=1, space="PSUM") as out_psum:
                        bout_sb = attall_pool.tile([128, D], F32, name="bout_sb", bufs=1)
                        nc.sync.dma_start(bout_sb[:], boutbc)
                        pools1 = (ex1_pool, rden1_pool, s1_psum, av1_psum, d1_psum)
                        for g in groups_h1:
                            emit_attn_group(qkv1, att1_sb, pools1, *g)
                        emit_a2a(att1_sb, 1, 1)
                        for g in groups_h0:
                            emit_attn_group(qkv1, att1_sb, pools1, *g)
                        emit_a2a(att1_sb, 1, 0)
                        emit_outproj(attall_pool, wout_pool, o_pool, out_psum, bout_sb, 0)
                        emit_outproj(attall_pool, wout_pool, o_pool, out_psum, bout_sb, 1)
    nc.compile()
    return nc


_CACHED_NC = None


def kernel(x, Wqkv, bqkv, Wout, bout):
    global _CACHED_NC
    x = np.asarray(x, dtype=np.float32)
    Wqkv = np.asarray(Wqkv, dtype=np.float32)
    bqkv = np.asarray(bqkv, dtype=np.float32)
    Wout = np.asarray(Wout, dtype=np.float32)
    bout = np.asarray(bout, dtype=np.float32)

    if _CACHED_NC is None:
        _CACHED_NC = _build()
    nc = _CACHED_NC

    xT = np.ascontiguousarray(x.reshape(NT, D).T)          # [D, NT]
    wq4 = Wqkv.reshape(D, 3, H, Dh)                        # col = which, head, dh
    bq4 = bqkv.reshape(3, H, Dh)
    kl = np.arange(128)[:, None]
    jl = np.arange(128)[None, :]
    masktri = (jl >= kl).astype(np.float32)

    in_maps = []
    for c in range(W):
        wshard = np.ascontiguousarray(
            wq4[:, :, HL * c:HL * c + HL, :].reshape(D, CQKV))
        bshard = np.ascontiguousarray(
            bq4[:, HL * c:HL * c + HL, :].reshape(CQKV))
        in_maps.append({
            "xT": xT, "wqkv": wshard, "bqkv": bshard,
            "wout": Wout, "masktri": masktri,
            "ones": np.ones((128, 128), np.float32),
            "bvbc": np.tile(bshard[2 * HL * 128:][None, :], (128, 1)),
            "boutbc": np.tile(bout[None, :], (128, 1)),
        })

    res = run_bass_kernel_spmd(nc, in_maps, core_ids=list(range(W)))
    # res[c]["out"] rows [(b*2+h)*TOKH ...) = tokens [h*HT + c*TOKH ...) of batch b
    full = np.empty((B, T, D), np.float32)
    for c in range(W):
        for b in range(B):
            for h in range(2):
                full[b, h * HT + c * TOKH:h * HT + (c + 1) * TOKH] = \
                    res.results[c]["out"][(b * 2 + h) * TOKH:(b * 2 + h + 1) * TOKH]
    return full



# revision 7
# speedup vs baseline: 1.3235x; 1.3235x over previous
"""Causal self-attention kernel for 8 Trainium2 NeuronCores.

Problem: B=2, T=2048, D=2048, H=16, Dh=128, fp32.
  qkv = x @ Wqkv + bqkv ; per-head causal attention ; out = att @ Wout + bout

Sharding (tensor parallel over heads + AllToAll before out_proj):
  Core c owns heads {2c, 2c+1}. Each core computes, for all 4096 tokens,
  Q^T/K^T (head-dim on partitions) and V (token-dim on partitions) for its
  two heads via the QKV projection with its 768-column shard of Wqkv, runs
  causal attention locally in S^T[k,q] layout, and produces att^T
  [256, 2048] per batch. Four AllToAlls (one per half-batch of tokens)
  redistribute from head-sharded to token-sharded; core c then projects its
  128-token slices with the full Wout.

All matmul operands are bf16 (fp32 PSUM accumulation) which halves HBM/
collective traffic and avoids the fp32r small-free-dim penalty; measured
end-to-end rel err ~4e-3. The softmax denominator is accumulated on the
vector engine in fp32 (one f32r ones-matmul per q-chunk broadcasts it
across partitions); normalization uses reciprocal_approx_fast. Batch-0
attention is interleaved with batch-1's projection; batch-0's output
projection (full Wout resident in SBUF, loaded once) is interleaved with
batch-1's attention so the PE never waits on collectives except the final
half-batch tail.
"""

import numpy as np
import ml_dtypes

import concourse.bass as bass
import concourse.mybir as mybir
import concourse.tile as tile
from concourse import bacc
from concourse.bass_utils import run_bass_kernel_spmd

B, T, D, H, Dh = 2, 2048, 2048, 16, 128
NT = B * T                  # 4096 tokens total
W = 8                       # cores
HL = H // W                 # 2 heads per core
KO = D // 128               # 16 contraction subtiles
TC = 512                    # token chunk for projection rhs
NTC_B = T // TC             # 4 chunks per batch
QC = 512                    # attention q-chunk
NQC = T // QC               # 4 q-chunks per batch
HT = T // 2                 # half-batch token span (one AllToAll each)
TOKH = HT // W              # 128 tokens per core per half-batch exchange
SCALE = 1.0 / float(np.sqrt(Dh))
NWU = 40                    # PE warmup matmuls

F32 = mybir.dt.float32
F32R = mybir.dt.float32r
BF16 = mybir.dt.bfloat16


def _build():
    nc = bacc.Bacc("TRN2", target_bir_lowering=False, debug=False,
                   enable_asserts=True, num_devices=W)
    xT = nc.dram_tensor("xT", [D, NT], BF16, kind="ExternalInput").ap()
    wqk = nc.dram_tensor("wqk", [4, 128, KO * 128], BF16, kind="ExternalInput").ap()
    wqv = nc.dram_tensor("wqv", [128, KO * 256], BF16, kind="ExternalInput").ap()
    bqkv = nc.dram_tensor("bqkv", [512], F32, kind="ExternalInput").ap()
    bvbc = nc.dram_tensor("bvbc", [128, HL * Dh], F32, kind="ExternalInput").ap()
    woutp = nc.dram_tensor("woutp", [128, KO * D], BF16, kind="ExternalInput").ap()
    boutbc = nc.dram_tensor("boutbc", [128, D], F32, kind="ExternalInput").ap()
    masktri = nc.dram_tensor("masktri", [128, 128], BF16, kind="ExternalInput").ap()
    ones = nc.dram_tensor("ones", [128, 128], F32, kind="ExternalInput").ap()
    # rows [(b*2+half)*TOKH ...): tokens [half*HT + c*TOKH ...) of batch b
    out = nc.dram_tensor("out", [B * 2 * TOKH, D], F32, kind="ExternalOutput").ap()

    xT_v = xT.rearrange("(ko p) t -> p ko t", p=128)

    with tile.TileContext(nc) as tc:
        with tc.tile_pool(name="persist", bufs=1) as persist, \
             tc.tile_pool(name="dram", bufs=1, space="DRAM") as dram_pool:
            ones_sb = persist.tile([128, 128], F32R)
            mask_sb = persist.tile([128, 128], BF16)
            bqk_sb = persist.tile([128, 4], F32)        # Q,K bias (col on partition)
            bv_sb = persist.tile([128, HL * Dh], F32)   # V bias pre-broadcast
            bout_sb = persist.tile([128, D], F32)

            nc.sync.dma_start(ones_sb[:], ones.bitcast(F32R))
            nc.sync.dma_start(mask_sb[:], masktri)
            nc.sync.dma_start(bqk_sb[:], bqkv.rearrange("(cc p) -> p cc", p=128))
            nc.sync.dma_start(bv_sb[:], bvbc)

            a2a_in = [[dram_pool.tile([W, HL * 128, TOKH], BF16, name=f"a2a_in{b}{h}")
                       for h in range(2)] for b in range(B)]
            a2a_out = [[dram_pool.tile([W, HL * 128, TOKH], BF16, name=f"a2a_out{b}{h}")
                        for h in range(2)] for b in range(B)]

            def alloc_qkv(pool):
                qT = pool.tile([128, HL, T], BF16, name="qT")
                kT = pool.tile([128, HL, T], BF16, name="kT")
                v = pool.tile([128, HL, T // 128, Dh], BF16, name="v")
                return qT, kT, v

            def emit_proj_chunk(qkv, wqk_sb, wqv_sb, x_pool, proj_psum, b, tci):
                """Project one 512-token chunk of batch b into (qT, kT, v)."""
                qT_sb, kT_sb, v_sb = qkv
                t0 = b * T + tci * TC
                x_sb = x_pool.tile([128, KO, TC], BF16, name="x_sb")
                nc.sync.dma_start(x_sb[:], xT_v[:, :, t0:t0 + TC])
                for cc in range(4):
                    ps = proj_psum.tile([128, TC], F32, name="proj_ps")
                    for ko in range(KO):
                        nc.tensor.matmul(
                            ps[:], wqk_sb[cc][:, ko, :], x_sb[:, ko, :],
                            start=(ko == 0), stop=(ko == KO - 1))
                    dest = qT_sb if cc < HL else kT_sb
                    hl = cc if cc < HL else cc - HL
                    nc.vector.tensor_scalar_add(
                        dest[:, hl, tci * TC:(tci + 1) * TC], ps[:],
                        bqk_sb[:, cc:cc + 1])
                for tb in range(TC // 128):
                    ps = proj_psum.tile([128, HL * Dh], F32, name="proj_ps")
                    for ko in range(KO):
                        nc.tensor.matmul(
                            ps[:], x_sb[:, ko, tb * 128:(tb + 1) * 128],
                            wqv_sb[:, ko, :], start=(ko == 0), stop=(ko == KO - 1))
                    idx = tci * (TC // 128) + tb
                    nc.vector.tensor_tensor(
                        v_sb[:, :, idx, :],
                        ps[:].rearrange("p (hl d) -> p hl d", hl=HL),
                        bv_sb[:].rearrange("p (hl d) -> p hl d", hl=HL),
                        mybir.AluOpType.add)

            def emit_attn_group(qkv, att_sb, pools, hl, qc):
                """One (head, q-chunk) attention group: S^T -> exp -> P^T V.

                k-blocks are processed in pairs sharing one 2-bank PSUM tile.
                The softmax denominator is accumulated on DVE in fp32 and
                broadcast across partitions by a single f32r ones-matmul.
                """
                qT_sb, kT_sb, v_sb = qkv
                ex_pool, esum_pool, rden_pool, s_psum, av_psum, d_psum = pools
                q0 = qc * QC
                nkb = (qc + 1) * (QC // 128)
                ps_av = av_psum.tile([128, QC], F32, name="ps_av")
                esum = esum_pool.tile([128, QC], F32R, name="esum")
                for kbp in range(nkb // 2):
                    kbs = (2 * kbp, 2 * kbp + 1)
                    os_ = [kb - qc * (QC // 128) for kb in kbs]
                    vss = [max(0, o) * 128 for o in os_]
                    ps_s2 = s_psum.tile([128, 2, QC], F32, name="ps_s2")
                    ex2 = ex_pool.tile([128, 2, QC], BF16, name="ex2")
                    for i, kb in enumerate(kbs):
                        nc.tensor.matmul(
                            ps_s2[:, i, vss[i]:], kT_sb[:, hl, kb * 128:(kb + 1) * 128],
                            qT_sb[:, hl, q0 + vss[i]:q0 + QC], start=True, stop=True)
                    if vss[0] == 0 and vss[1] == 0:
                        nc.scalar.activation(
                            ex2[:], ps_s2[:], mybir.ActivationFunctionType.Exp,
                            scale=SCALE)
                    else:
                        for i in range(2):
                            nc.scalar.activation(
                                ex2[:, i, vss[i]:], ps_s2[:, i, vss[i]:],
                                mybir.ActivationFunctionType.Exp, scale=SCALE)
                    for i, kb in enumerate(kbs):
                        if os_[i] >= 0:
                            nc.vector.tensor_tensor(
                                ex2[:, i, vss[i]:vss[i] + 128],
                                ex2[:, i, vss[i]:vss[i] + 128], mask_sb[:],
                                mybir.AluOpType.mult)
                        nc.tensor.matmul(
                            ps_av[:, vss[i]:], v_sb[:, hl, kb, :], ex2[:, i, vss[i]:],
                            start=(kb == 0), stop=(kb == nkb - 1))
                    # fp32 denominator partial sums on DVE
                    if kbp == 0:
                        if vss[1] == 0:
                            nc.vector.tensor_tensor(
                                esum[:], ex2[:, 0, :], ex2[:, 1, :],
                                mybir.AluOpType.add)
                        else:  # qc == 0: diagonal pair (vss 0, 128)
                            nc.vector.tensor_scalar_add(esum[:], ex2[:, 0, :], 0.0)
                            nc.vector.tensor_tensor(
                                esum[:, 128:], esum[:, 128:], ex2[:, 1, 128:],
                                mybir.AluOpType.add)
                    else:
                        for i in range(2):
                            nc.vector.tensor_tensor(
                                esum[:, vss[i]:], esum[:, vss[i]:],
                                ex2[:, i, vss[i]:], mybir.AluOpType.add)
                # broadcast denominator over partitions via ones-matmul (f32r)
                ps_dbc = d_psum.tile([128, QC], F32, name="ps_dbc")
                nc.tensor.matmul(ps_dbc[:], ones_sb[:], esum[:],
                                 start=True, stop=True)
                rden = rden_pool.tile([128, QC], F32, name="rden")
                nc.vector.reciprocal_approx_fast(rden[:], ps_dbc[:])
                nc.vector.tensor_tensor(
                    att_sb[:, hl, q0:q0 + QC], ps_av[:], rden[:],
                    mybir.AluOpType.mult)

            def emit_a2a(att_sb, b, half):
                for r in range(W):
                    nc.gpsimd.dma_start(
                        a2a_in[b][half][r].rearrange("(hl p) t -> p hl t", hl=HL, p=128),
                        att_sb[:, :, half * HT + r * TOKH:half * HT + (r + 1) * TOKH])
                nc.gpsimd.collective_compute(
                    "AllToAll", mybir.AluOpType.bypass,
                    replica_groups=[list(range(W))],
                    ins=[a2a_in[b][half][:].opt()], outs=[a2a_out[b][half][:].opt()])

            # attention group order: heavy half (qc 3,2) first so each
            # half-batch AllToAll fires as early as possible
            groups_h1 = [(hl, qc) for qc in (3, 2) for hl in range(HL)]
            groups_h0 = [(hl, qc) for qc in (1, 0) for hl in range(HL)]

            with tc.tile_pool(name="qkv1_pool", bufs=1) as qkv1_pool, \
                 tc.tile_pool(name="att1_pool", bufs=1) as att1_pool:
                qkv1 = alloc_qkv(qkv1_pool)
                att1_sb = att1_pool.tile([128, HL, T], BF16)

                with tc.tile_pool(name="wq_pool", bufs=1) as wq_pool, \
                     tc.tile_pool(name="x_pool", bufs=2) as x_pool, \
                     tc.tile_pool(name="qkv0_pool", bufs=1) as qkv0_pool, \
                     tc.tile_pool(name="att0_pool", bufs=1) as att0_pool, \
                     tc.tile_pool(name="ex0_pool", bufs=3) as ex0_pool, \
                     tc.tile_pool(name="esum0_pool", bufs=2) as esum0_pool, \
                     tc.tile_pool(name="rden0_pool", bufs=2) as rden0_pool, \
                     tc.tile_pool(name="proj_psum", bufs=2, space="PSUM") as proj_psum, \
                     tc.tile_pool(name="s0_psum", bufs=2, space="PSUM") as s0_psum, \
                     tc.tile_pool(name="av0_psum", bufs=1, space="PSUM") as av0_psum, \
                     tc.tile_pool(name="d0_psum", bufs=1, space="PSUM") as d0_psum:
                    qkv0 = alloc_qkv(qkv0_pool)
                    att0_sb = att0_pool.tile([128, HL, T], BF16)

                    # PE warmup on the ones tile while the first DMAs stream
                    # (reuses the proj_ps pool slot to avoid an extra PSUM bank)
                    wu_ps = proj_psum.tile([128, 512], F32, name="proj_ps")
                    for _ in range(NWU):
                        nc.tensor.matmul(wu_ps[:, 0:128], ones_sb[:], ones_sb[:],
                                         start=True, stop=True)

                    wqk_sb = [wq_pool.tile([128, KO, 128], BF16, name=f"wqk{cc}",
                                           bufs=1) for cc in range(4)]
                    wqv_sb = wq_pool.tile([128, KO, 256], BF16, name="wqv", bufs=1)
                    for cc in range(4):
                        nc.sync.dma_start(
                            wqk_sb[cc][:],
                            wqk[cc].rearrange("p (ko j) -> p ko j", ko=KO))
                    nc.sync.dma_start(
                        wqv_sb[:], wqv.rearrange("p (ko j) -> p ko j", ko=KO))

                    pools0 = (ex0_pool, esum0_pool, rden0_pool,
                              s0_psum, av0_psum, d0_psum)
                    # batch-0 projection
                    for tci in range(NTC_B):
                        emit_proj_chunk(qkv0, wqk_sb, wqv_sb, x_pool, proj_psum,
                                        0, tci)
                    # batch-1 projection interleaved with batch-0 attention
                    groups0 = groups_h1 + groups_h0
                    for i in range(NTC_B):
                        emit_proj_chunk(qkv1, wqk_sb, wqv_sb, x_pool, proj_psum,
                                        1, i)
                        emit_attn_group(qkv0, att0_sb, pools0, *groups0[2 * i])
                        emit_attn_group(qkv0, att0_sb, pools0, *groups0[2 * i + 1])
                        if i == 1:
                            emit_a2a(att0_sb, 0, 1)
                    emit_a2a(att0_sb, 0, 0)

                # phase B: batch-1 attention + all output projections.
                # wout/attall SBUF reuses the freed phase-A space; the wout DMA
                # starts as soon as the last phase-A reader of that space is done.
                with tc.tile_pool(name="wout_pool", bufs=1) as wout_pool, \
                     tc.tile_pool(name="attall_pool", bufs=1) as attall_pool, \
                     tc.tile_pool(name="o_pool", bufs=3) as o_pool, \
                     tc.tile_pool(name="ex1_pool", bufs=3) as ex1_pool, \
                     tc.tile_pool(name="esum1_pool", bufs=2) as esum1_pool, \
                     tc.tile_pool(name="rden1_pool", bufs=2) as rden1_pool, \
                     tc.tile_pool(name="s1_psum", bufs=2, space="PSUM") as s1_psum, \
                     tc.tile_pool(name="av1_psum", bufs=2, space="PSUM") as av1_psum, \
                     tc.tile_pool(name="d1_psum", bufs=1, space="PSUM") as d1_psum, \
                     tc.tile_pool(name="out_psum", bufs=1, space="PSUM") as out_psum:
                    nc.sync.dma_start(bout_sb[:], boutbc)
                    wout_sb = wout_pool.tile([128, KO, D], BF16)
                    woutp_v = woutp.rearrange("p (ko c) -> p ko c", ko=KO)

                    attall = {}

                    def load_attall(b, h):
                        sb = attall_pool.tile([128, KO, TOKH], BF16,
                                              name=f"attall{b}{h}", bufs=1)
                        nc.sync.dma_start(
                            sb[:],
                            a2a_out[b][h][:].rearrange(
                                "r (x p) t -> p (r x) t", x=HL, p=128))
                        attall[(b, h)] = sb

                    def op_piece(b, h, colc):
                        ps_o = out_psum.tile([128, 512], F32, name="ps_o")
                        for ko in range(KO):
                            nc.tensor.matmul(
                                ps_o[:], attall[(b, h)][:, ko, :],
                                wout_sb[:, ko, colc * 512:(colc + 1) * 512],
                                start=(ko == 0), stop=(ko == KO - 1))
                        o_sb = o_pool.tile([128, 512], F32, name="o_sb")
                        nc.vector.tensor_tensor(
                            o_sb[:], ps_o[:],
                            bout_sb[:, colc * 512:(colc + 1) * 512],
                            mybir.AluOpType.add)
                        nc.sync.dma_start(
                            out[(b * 2 + h) * TOKH:(b * 2 + h + 1) * TOKH,
                                colc * 512:(colc + 1) * 512],
                            o_sb[:])

                    pools1 = (ex1_pool, esum1_pool, rden1_pool,
                              s1_psum, av1_psum, d1_psum)
                    g = lambda n: emit_attn_group(qkv1, att1_sb, pools1,
                                                  *(groups_h1 + groups_h0)[n])
                    load_attall(0, 1)          # a2a(0,1) completed in phase A
                    # wout split over 4 DMAs to spread across hardware queues
                    for kq in range(4):
                        nc.sync.dma_start(wout_sb[:, 4 * kq:4 * kq + 4, :],
                                          woutp_v[:, 4 * kq:4 * kq + 4, :])
                    g(0); g(1)                 # qc3
                    load_attall(0, 0)          # a2a(0,0) completes early here
                    g(2); op_piece(0, 1, 0)
                    g(3); op_piece(0, 1, 1)
                    emit_a2a(att1_sb, 1, 1)
                    g(4); op_piece(0, 1, 2)
                    g(5); op_piece(0, 1, 3)
                    op_piece(0, 0, 0)
                    g(6); op_piece(0, 0, 1)
                    g(7); op_piece(0, 0, 2)
                    emit_a2a(att1_sb, 1, 0)
                    op_piece(0, 0, 3)
                    load_attall(1, 1)
                    for colc in range(4):
                        op_piece(1, 1, colc)
                    load_attall(1, 0)
                    for colc in range(4):
                        op_piece(1, 0, colc)
    nc.compile()
    return nc


_CACHED_NC = None


def kernel(x, Wqkv, bqkv, Wout, bout):
    global _CACHED_NC
    x = np.asarray(x, dtype=np.float32)
    Wqkv = np.asarray(Wqkv, dtype=np.float32)
    bqkv = np.asarray(bqkv, dtype=np.float32)
    Wout = np.asarray(Wout, dtype=np.float32)
    bout = np.asarray(bout, dtype=np.float32)

    if _CACHED_NC is None:
        _CACHED_NC = _build()
    nc = _CACHED_NC

    bf16 = ml_dtypes.bfloat16
    xT = np.ascontiguousarray(x.reshape(NT, D).T).astype(bf16)   # [D, NT]
    wq4 = Wqkv.reshape(D, 3, H, Dh)
    bq3 = bqkv.reshape(3, H, Dh)
    woutp = np.ascontiguousarray(
        Wout.reshape(KO, 128, D).transpose(1, 0, 2).reshape(128, KO * D)
    ).astype(bf16)
    boutbc = np.ascontiguousarray(np.tile(bout[None, :], (128, 1)))
    kl = np.arange(128)[:, None]
    jl = np.arange(128)[None, :]
    masktri = (jl >= kl).astype(bf16)
    onesm = np.ones((128, 128), np.float32)

    in_maps = []
    for c in range(W):
        wsh = np.ascontiguousarray(
            wq4[:, :, HL * c:HL * c + HL, :].reshape(D, 3 * HL * Dh))
        bsh = np.ascontiguousarray(
            bq3[:, HL * c:HL * c + HL, :].reshape(3 * HL * Dh))
        wqk = np.ascontiguousarray(
            wsh[:, :512].reshape(KO, 128, 4, 128).transpose(2, 1, 0, 3)
            .reshape(4, 128, KO * 128)).astype(bf16)
        wqv = np.ascontiguousarray(
            wsh[:, 512:].reshape(KO, 128, 256).transpose(1, 0, 2)
            .reshape(128, KO * 256)).astype(bf16)
        in_maps.append({
            "xT": xT, "wqk": wqk, "wqv": wqv,
            "bqkv": np.ascontiguousarray(bsh[:512]),
            "bvbc": np.ascontiguousarray(np.tile(bsh[512:][None, :], (128, 1))),
            "woutp": woutp, "boutbc": boutbc,
            "masktri": masktri, "ones": onesm,
        })

    res = run_bass_kernel_spmd(nc, in_maps, core_ids=list(range(W)))
    # res[c]["out"] rows [(b*2+h)*TOKH ...) = tokens [h*HT + c*TOKH ...) of batch b
    full = np.empty((B, T, D), np.float32)
    for c in range(W):
        for b in range(B):
            for h in range(2):
                full[b, h * HT + c * TOKH:h * HT + (c + 1) * TOKH] = \
                    res.results[c]["out"][(b * 2 + h) * TOKH:(b * 2 + h + 1) * TOKH]
    return full
